# revision 6
# baseline (speedup 1.0000x reference)
"""Trainium2 Bass kernel for nn_ExtSummarizer (B=512, S=100, H=768).

Math (per batch b, mask==1, true_dim==S):
  off[i] = s_i . v,  v = W_rel d + W_cont^T,  d = mean_i s_i   (host, fp32)
  q = sigmoid(s W_sim s^T + off[:,None] + b)
  sv[j] = sum_i q[i,j];  solve (I - lam*q*diag(1/sv)) x = y,  y = 1/S
  score = (1-lam) x

Device algorithm (transposed formulation, fp8 e4m3 matmuls):
  - mm1: yt[h',r] = sum_h (WS*W_sim)[h,h'] s[r,h] via fp8 DoubleRow
    (K=256/instr), fp32 PSUM, drained to fp8 yt (ACT/DVE alternating).
  - per 4-batch PSUM block (contiguous 400 cols of one bank): 12 fp8
    DoubleRow sim matmuls (single start on the first — per-element
    has_written gives each batch's columns overwrite-then-accumulate),
    then ONE K=1 fp16 matmul adds off over all 400 cols (stop=True).
    One ACT sigmoid -> qT fp16; DVE reduce -> sv (fp16); reciprocal;
    NT = lam*qT/sv (per-partition).  One group-wide DMA-xbar-transpose
    NT -> N after all blocks.
  - solve x = sum_{k<24} N^k z: 3 uniform doubling levels, each ONE
    matmul per batch: [N^2 | Nz+...] = N @ [N|z]; (N^2)^T via ONE
    group-wide DMA transpose per level.  Then w1=N^8 z3, w2=N^8 w1,
    x = z3+w1+w2.  fp16 operands.
  - N/NT stored in 128-col blocks per batch (z in col 100); garbage in
    unused rows/cols is never consumed by arithmetic.
  - emission interleaves solve(g) with mm1/phase2(g+1); 4 groups of 16.

Sharding: pure data parallel, 64 batches per core, 8 cores.
"""

import numpy as np
import ml_dtypes

B, S, H = 512, 100, 768
NCORES = 8
BC = B // NCORES          # 64 batches per core
ROWS = BC * S             # 6400 rows per core
LAMB = 0.8
GSIZES = [16, 16, 16, 16]
NGRP = len(GSIZES)
GB0 = 16                  # max group size (tile sizing)
GROWS = GB0 * S           # 1600 rows max per group
HC = H // 128             # 6 k-chunks
HP = HC // 2              # 3 DoubleRow k-pairs
NT = 400                  # mm1 moving-dim tile
BST = 128                 # N/NT per-batch block stride (z at col S)
NLEV = 3                  # uniform doubling levels
WS = 16.0                 # fp8 scale on W_sim / off
Z0 = (1.0 - LAMB) / S
PADC = 128                # lhsT column width (FWL)
SPAD = GROWS + PADC
TRP = 112                 # DMA-transpose source rows (16-multiple >= S)
E4NP = ml_dtypes.float8_e4m3
ILV_SKIP = 0
ILV_RATE = 1

_CACHE = {}


def _get_nc(loop_n=1):
    key = ("nc", loop_n)
    if key in _CACHE:
        return _CACHE[key]

    import contextlib

    import concourse.mybir as mybir
    import concourse.tile as tile
    from concourse import bacc
    from concourse.bass import ts

    fp8 = mybir.dt.float8e4
    fp16 = mybir.dt.float16
    fp32 = mybir.dt.float32
    AF = mybir.ActivationFunctionType
    OP = mybir.AluOpType
    X = mybir.AxisListType.X
    PM = mybir.MatmulPerfMode.DoubleRow

    nc = bacc.Bacc(trn_type="TRN2", target_bir_lowering=False, debug=False)

    sent8 = nc.dram_tensor("sent8", [128, NGRP, HC, GROWS], fp8,
                           kind="ExternalInput")
    wsim8 = nc.dram_tensor("wsim8", [128, HC, H], fp8, kind="ExternalInput")
    off16h = nc.dram_tensor("off16h", [1, ROWS], fp16, kind="ExternalInput")
    onesr16 = nc.dram_tensor("onesr16", [1, PADC], fp16, kind="ExternalInput")
    bvec32 = nc.dram_tensor("bvec32", [S, 1], fp32, kind="ExternalInput")
    out32 = nc.dram_tensor("out32", [S, BC], fp32, kind="ExternalOutput")

    NW = GB0 * BST            # N/NT tile width (max)
    GOFF = [sum(GSIZES[:i]) for i in range(NGRP)]

    with tile.TileContext(nc) as tc:
        loop_cm = tc.For_i(0, loop_n, 1) if loop_n > 1 else contextlib.nullcontext()
        with (
            loop_cm,
            tc.tile_pool(name="const", bufs=1) as const,
            tc.tile_pool(name="sentT_p", bufs=2) as sentT_p,
            tc.tile_pool(name="yt_p", bufs=2) as yt_p,
            tc.tile_pool(name="grp_p", bufs=2) as grp_p,
            tc.tile_pool(name="solve_p", bufs=5) as solve_p,
            tc.tile_pool(name="small", bufs=4) as small,
            tc.tile_pool(name="psmm", bufs=2, space="PSUM") as psmm,
            tc.tile_pool(name="psb2", bufs=2, space="PSUM") as psb2_p,
            tc.tile_pool(name="pssq", bufs=3, space="PSUM") as pssq_p,
            tc.tile_pool(name="pssv", bufs=1, space="PSUM") as pssv_p,
        ):
            wsim_sb = const.tile([128, HC, H], fp8)
            nc.sync.dma_start(wsim_sb[:], wsim8.ap())
            off_sb = const.tile([1, ROWS], fp16)
            nc.sync.dma_start(off_sb[:], off16h.ap())
            onesr_sb = const.tile([1, PADC], fp16)
            nc.sync.dma_start(onesr_sb[:], onesr16.ap())
            bvec_sb = const.tile([S, 1], fp32)
            nc.sync.dma_start(bvec_sb[:], bvec32.ap())

            st = {}          # per-group live tiles
            par = [0]        # drain engine parity

            def alt_copy(dst, src):
                if par[0] % 2 == 0:
                    nc.scalar.copy(dst, src)
                else:
                    nc.vector.tensor_copy(dst, src)
                par[0] += 1

            def load_thunk(g):
                gb = GSIZES[g]
                grows = gb * S
                def t():
                    sentT = sentT_p.tile([128, HC, SPAD], fp8, tag="sentT",
                                         name=f"sentT{g}")
                    st["sentT", g] = sentT
                    nc.gpsimd.memset(
                        sentT[:, :, grows : grows + PADC], 0.0
                    )
                    nc.sync.dma_start(
                        out=sentT[:, :, 0:grows],
                        in_=sent8.ap()[:, g, :, :],
                    )
                return [t]

            def mm1_thunks(g):
                gb = GSIZES[g]
                grows = gb * S
                def start():
                    yt = yt_p.tile([128, HC, SPAD], fp8, tag="yt",
                                   name=f"yt{g}")
                    st["yt", g] = yt
                    nc.gpsimd.memset(yt[:, :, grows : grows + PADC], 0.0)
                out = [start]

                def tile_t(n, m):
                    def t():
                        sentT = st["sentT", g]
                        yt = st["yt", g]
                        psy = psmm.tile([128, 512], fp32, tag="mm",
                                        name=f"psy{g}_{n}_{m}")
                        for tt in range(HP):
                            nc.tensor.matmul(
                                psy[:, :NT],
                                wsim_sb[:, 2 * tt : 2 * tt + 2,
                                        m * 128 : (m + 1) * 128],
                                sentT[:, 2 * tt : 2 * tt + 2, ts(n, NT)],
                                start=(tt == 0),
                                stop=(tt == HP - 1),
                                perf_mode=PM,
                            )
                        alt_copy(yt[:, m, ts(n, NT)], psy[:, :NT])
                    return t

                for n in range(grows // NT):
                    for m in range(HC):
                        out.append(tile_t(n, m))
                return out

            def ph2_thunks(g):
                gb = GSIZES[g]
                r0g = GOFF[g] * S
                def start():
                    N_cur = solve_p.tile([128, NW], fp16, tag="Nall",
                                         name=f"N0g{g}")
                    NT_cur = solve_p.tile([128, NW], fp16, tag="NTall",
                                          name=f"NT0g{g}")
                    st["N", g] = N_cur
                    st["NT", g] = NT_cur
                    st["svg", g] = grp_p.tile([S, GB0], fp16, tag="svg",
                                              name=f"svg{g}")
                    st["rg", g] = grp_p.tile([S, GB0], fp32, tag="rg",
                                             name=f"rg{g}")
                out = [start]

                def blk_t(blk):
                    def t():
                        sentT = st["sentT", g]
                        yt = st["yt", g]
                        NT_cur = st["NT", g]
                        svg, rg = st["svg", g], st["rg", g]
                        psb = psb2_p.tile([128, 512], fp32, tag="sim",
                                          name=f"sim{g}_{blk}")
                        # 12 DoubleRow matmuls into one bank, contiguous
                        # 100-col slots; single start clears the bank,
                        # per-element has_written handles the rest.
                        for q in range(4):
                            bl = blk * 4 + q
                            r0 = bl * S
                            dst = psb[:, q * S : q * S + S]
                            for tt in range(HP):
                                nc.tensor.matmul(
                                    dst,
                                    sentT[:, 2 * tt : 2 * tt + 2,
                                          r0 : r0 + PADC],
                                    yt[:, 2 * tt : 2 * tt + 2, r0 : r0 + S],
                                    start=(q == 0 and tt == 0),
                                    stop=False,
                                    perf_mode=PM,
                                )
                        # one K=1 off matmul over all 4 batches (400 cols)
                        nc.tensor.matmul(
                            psb[:, 0 : 4 * S],
                            onesr_sb[:],
                            off_sb[0:1, r0g + blk * 4 * S : r0g + (blk + 1) * 4 * S],
                            start=False,
                            stop=True,
                        )
                        qT4 = small.tile([S, 4 * S], fp16, tag="qT",
                                         name=f"qT{g}_{blk}")
                        nc.scalar.activation(
                            qT4[:],
                            psb[:S, 0 : 4 * S],
                            AF.Sigmoid,
                            bias=bvec_sb[:, 0:1],
                            scale=1.0 / WS,
                        )
                        with nc.allow_low_precision(reason="sv ~ 50, fp16 ok"):
                            nc.vector.reduce_sum(
                                out=svg[:, blk * 4 : blk * 4 + 4],
                                in_=qT4[:].rearrange("p (f w) -> p f w", w=S),
                                axis=X,
                            )
                        with nc.allow_low_precision(reason="1/sv fp16 ok"):
                            nc.vector.reciprocal(
                                rg[:, blk * 4 : blk * 4 + 4],
                                svg[:, blk * 4 : blk * 4 + 4],
                            )
                        for q in range(4):
                            bl = blk * 4 + q
                            nc.vector.tensor_scalar(
                                out=NT_cur[0:S, bl * BST : bl * BST + S],
                                in0=qT4[:, q * S : (q + 1) * S],
                                scalar1=rg[:, bl : bl + 1],
                                scalar2=LAMB,
                                op0=OP.mult,
                                op1=OP.mult,
                            )
                    return t

                out.extend(blk_t(blk) for blk in range(gb // 4))

                def transp():
                    # one group-wide transpose NT -> N
                    nc.sync.dma_start_transpose(
                        st["N", g][:, 0 : gb * BST].rearrange(
                            "p (n w) -> p n w", w=BST
                        )[:, :, 0:TRP],
                        st["NT", g][0:TRP, 0 : gb * BST],
                    )
                out.append(transp)

                def zset():
                    nc.vector.memset(
                        st["N", g][0:S, 0 : gb * BST].rearrange(
                            "p (n w) -> p n w", w=BST
                        )[:, :, S : S + 1],
                        Z0,
                    )
                out.append(zset)
                return out

            def solve_thunks(g):
                gb = GSIZES[g]
                packs = [(p, 4) for p in range(0, gb, 4)]
                out = []

                def lvl_start(j):
                    def t():
                        N_nxt = solve_p.tile([128, NW], fp16, tag="Nall",
                                             name=f"N{j + 1}g{g}")
                        NT_nxt = solve_p.tile([128, NW], fp16, tag="NTall",
                                              name=f"NT{j + 1}g{g}")
                        st["Nn", g] = N_nxt
                        st["NTn", g] = NT_nxt
                    return t

                def lvl_pack(j, p0, np_):
                    def t():
                        N_cur, NT_cur = st["N", g], st["NT", g]
                        N_nxt = st["Nn", g]
                        sq = pssq_p.tile([128, 512], fp32, tag="sq",
                                         name=f"sq{g}_{j}_{p0}")
                        for i in range(np_):
                            b = p0 + i
                            nc.tensor.matmul(
                                sq[:, i * BST : i * BST + S + 1],
                                NT_cur[0:S, b * BST : b * BST + PADC],
                                N_cur[0:S, b * BST : b * BST + S + 1],
                                start=True,
                                stop=True,
                            )
                        sq3 = sq[:S, :].rearrange("p (n w) -> p n w", w=BST)
                        dst3 = N_nxt[0:S, p0 * BST : (p0 + np_) * BST].rearrange(
                            "p (n w) -> p n w", w=BST
                        )
                        cur_z = N_cur[0:S, p0 * BST : (p0 + np_) * BST].rearrange(
                            "p (n w) -> p n w", w=BST
                        )[:, :, S : S + 1]
                        alt_copy(dst3[:, :, 0:S], sq3[:, 0:np_, 0:S])
                        nc.vector.tensor_tensor(
                            out=dst3[:, :, S : S + 1],
                            in0=sq3[:, 0:np_, S : S + 1],
                            in1=cur_z,
                            op=OP.add,
                        )
                    return t

                def lvl_transp(j):
                    def t():
                        # one group-wide transpose N_nxt -> NT_nxt
                        nc.sync.dma_start_transpose(
                            st["NTn", g][:, 0 : gb * BST].rearrange(
                                "p (n w) -> p n w", w=BST
                            )[:, :, 0:TRP],
                            st["Nn", g][0:TRP, 0 : gb * BST],
                        )
                    return t

                def lvl_end(j):
                    def t():
                        st["N", g] = st["Nn", g]
                        st["NT", g] = st["NTn", g]
                    return t

                for j in range(NLEV):
                    out.append(lvl_start(j))
                    for p0, np_ in packs:
                        out.append(lvl_pack(j, p0, np_))
                    out.append(lvl_transp(j))
                    out.append(lvl_end(j))

                # matvec rounds: w1 = A z3, w2 = A w1, x = z3 + w1 + w2
                def mv1():
                    N_cur, NT_cur = st["N", g], st["NT", g]
                    w1p = pssv_p.tile([128, 512], fp32, tag="sv",
                                      name=f"w1p{g}")
                    for bl in range(gb):
                        nc.tensor.matmul(
                            w1p[:, bl : bl + 1],
                            NT_cur[0:S, bl * BST : bl * BST + PADC],
                            N_cur[0:S, bl * BST + S : bl * BST + S + 1],
                            start=True,
                            stop=True,
                        )
                    w1 = grp_p.tile([S, GB0], fp16, tag="w1",
                                    name=f"w1{g}")
                    st["w1", g] = w1
                    nc.vector.tensor_copy(w1[:, 0:gb], w1p[:S, 0:gb])
                    xt = grp_p.tile([S, GB0], fp32, tag="xt",
                                    name=f"xt{g}")
                    st["xt", g] = xt
                    zc = N_cur[0:S, 0 : gb * BST].rearrange(
                        "p (n w) -> p n w", w=BST
                    )
                    nc.vector.tensor_tensor(
                        out=xt[:, 0:gb].rearrange("p (n w) -> p n w", w=1),
                        in0=zc[:, :, S : S + 1],
                        in1=w1[:, 0:gb].rearrange("p (n w) -> p n w", w=1),
                        op=OP.add,
                    )
                out.append(mv1)

                def mv2():
                    NT_cur = st["NT", g]
                    w1 = st["w1", g]
                    xt = st["xt", g]
                    w2p = pssv_p.tile([128, 512], fp32, tag="sv",
                                      name=f"w2p{g}")
                    for bl in range(gb):
                        nc.tensor.matmul(
                            w2p[:, bl : bl + 1],
                            NT_cur[0:S, bl * BST : bl * BST + PADC],
                            w1[:, bl : bl + 1],
                            start=True,
                            stop=True,
                        )
                    xg = grp_p.tile([S, GB0], fp32, tag="xg",
                                    name=f"xg{g}")
                    nc.vector.tensor_tensor(
                        out=xg[:, 0:gb], in0=w2p[:S, 0:gb],
                        in1=xt[:, 0:gb], op=OP.add
                    )
                    nc.sync.dma_start(
                        out=out32.ap()[:, GOFF[g] : GOFF[g] + gb],
                        in_=xg[:, 0:gb],
                    )
                out.append(mv2)
                return out

            # ---- interleaved emission
            from collections import deque

            pending = deque()
            for g in range(NGRP):
                stream_a = []
                if g == 0:
                    stream_a += load_thunk(0)
                if g + 1 < NGRP:
                    stream_a += load_thunk(g + 1)
                stream_a += mm1_thunks(g)
                stream_a += ph2_thunks(g)
                for k, a in enumerate(stream_a):
                    a()
                    if k >= ILV_SKIP and pending:
                        for _ in range(ILV_RATE):
                            if pending:
                                pending.popleft()()
                pending.extend(solve_thunks(g))
            while pending:
                pending.popleft()()

    nc.compile()
    _CACHE[key] = nc
    return nc


def _prep(inputs):
    sent = np.ascontiguousarray(np.asarray(inputs["sent_vec"], dtype=np.float32))
    # [128, NGRP, HC, GROWS] per core, contiguous 1600B runs
    s_r = sent.reshape(NCORES, NGRP, GROWS, HC, 128)
    sent8 = np.ascontiguousarray(s_r.transpose(0, 4, 1, 3, 2)).astype(E4NP)
    W_rel = np.asarray(inputs["W_rel"], dtype=np.float32)
    W_cont = np.asarray(inputs["W_cont"], dtype=np.float32).reshape(H)
    sb = sent.reshape(B, S, H)
    d = sb.mean(axis=1)
    v = d @ W_rel.T + W_cont[None, :]
    off = np.matmul(sb, v[:, :, None])[:, :, 0]
    off16 = (off * WS).astype(np.float16).reshape(NCORES, 1, ROWS)
    W_sim = np.asarray(inputs["W_sim"], dtype=np.float32) * WS
    wsim8 = np.ascontiguousarray(
        W_sim.reshape(HC, 128, H).transpose(1, 0, 2)
    ).astype(E4NP)
    bval = float(np.asarray(inputs["b_matrix"]).reshape(-1)[0])
    onesr = np.ones((1, PADC), np.float16)
    bvec = np.full((S, 1), bval, np.float32)
    return [
        {
            "sent8": np.ascontiguousarray(sent8[i]),
            "wsim8": wsim8,
            "off16h": np.ascontiguousarray(off16[i]),
            "onesr16": onesr,
            "bvec32": bvec,
        }
        for i in range(NCORES)
    ]


def _run(in_maps, trace=False, **kw):
    from concourse.bass_utils import run_bass_kernel_spmd

    nc = _get_nc()
    return run_bass_kernel_spmd(nc, in_maps, list(range(NCORES)), trace=trace, **kw)


def kernel(**inputs):
    in_maps = _prep(inputs)
    res = _run(in_maps)
    out = np.concatenate([r["out32"].T for r in res.results], axis=0)
    return np.ascontiguousarray(out, dtype=np.float32)


if __name__ == "__main__":
    _get_nc()
    print("build ok")


# revision 7
# speedup vs baseline: 1.2878x; 1.2878x over previous
"""Trainium2 Bass kernel for nn_ExtSummarizer (B=512, S=100, H=768).

Math (per batch b, mask==1, true_dim==S):
  off[i] = s_i . v,  v = W_rel d + W_cont^T,  d = mean_i s_i   (host, fp32)
  q = sigmoid(s W_sim s^T + off[:,None] + b)
  sv[j] = sum_i q[i,j];  solve (I - lam*q*diag(1/sv)) x = y,  y = 1/S
  score = (1-lam) x

Device algorithm (transposed formulation, fp8 e4m3 matmuls):
  - mm1: yt[h',r] = sum_h (WS*W_sim)[h,h'] s[r,h] via fp8 DoubleRow
    (K=256/instr), fp32 PSUM, drained to fp8 yt (ACT/DVE alternating).
  - per 4-batch PSUM block (contiguous 400 cols of one bank): 12 fp8
    DoubleRow sim matmuls (single start on the first — per-element
    has_written gives each batch's columns overwrite-then-accumulate),
    then ONE K=1 fp16 matmul adds off over all 400 cols (stop=True).
    One ACT sigmoid -> qT fp16; DVE reduce -> sv (fp16); reciprocal;
    NT = lam*qT/sv (per-partition).  One group-wide DMA-xbar-transpose
    NT -> N after all blocks.
  - solve x = sum_{k<24} N^k z: 3 uniform doubling levels, each ONE
    matmul per batch: [N^2 | Nz+...] = N @ [N|z]; (N^2)^T via ONE
    group-wide DMA transpose per level.  Then w1=N^8 z3, w2=N^8 w1,
    x = z3+w1+w2.  fp16 operands.
  - N/NT stored in 128-col blocks per batch (z in col 100); garbage in
    unused rows/cols is never consumed by arithmetic.
  - emission interleaves solve(g) with mm1/phase2(g+1); 4 groups of 16.

Sharding: pure data parallel, 64 batches per core, 8 cores.
"""

import numpy as np
import ml_dtypes

B, S, H = 512, 100, 768
NCORES = 8
BC = B // NCORES          # 64 batches per core
ROWS = BC * S             # 6400 rows per core
LAMB = 0.8
GSIZES = [16, 16, 16, 16]
NGRP = len(GSIZES)
GB0 = 16                  # max group size (tile sizing)
GROWS = GB0 * S           # 1600 rows max per group
HC = H // 128             # 6 k-chunks
HP = HC // 2              # 3 DoubleRow k-pairs
NT = 400                  # mm1 moving-dim tile
BST = 128                 # N/NT per-batch block stride (z at col S)
NLEV = 3                  # uniform doubling levels
WS = 16.0                 # fp8 scale on W_sim / off
Z0 = (1.0 - LAMB) / S
PADC = 128                # lhsT column width (FWL)
SPAD = GROWS + PADC
TRP = 112                 # DMA-transpose source rows (16-multiple >= S)
E4NP = ml_dtypes.float8_e4m3
ILV_SKIP = 0
ILV_RATE = 1

_CACHE = {}


def _get_nc(loop_n=1):
    key = ("nc", loop_n)
    if key in _CACHE:
        return _CACHE[key]

    import contextlib

    import concourse.mybir as mybir
    import concourse.tile as tile
    from concourse import bacc
    from concourse.bass import ts

    fp8 = mybir.dt.float8e4
    fp16 = mybir.dt.float16
    fp32 = mybir.dt.float32
    AF = mybir.ActivationFunctionType
    OP = mybir.AluOpType
    X = mybir.AxisListType.X
    PM = mybir.MatmulPerfMode.DoubleRow

    nc = bacc.Bacc(trn_type="TRN2", target_bir_lowering=False, debug=False)

    sent8 = nc.dram_tensor("sent8", [128, NGRP, HC, GROWS], fp8,
                           kind="ExternalInput")
    wsim8 = nc.dram_tensor("wsim8", [128, HC, H], fp8, kind="ExternalInput")
    off16h = nc.dram_tensor("off16h", [1, ROWS], fp16, kind="ExternalInput")
    onesr16 = nc.dram_tensor("onesr16", [1, PADC], fp16, kind="ExternalInput")
    bvec32 = nc.dram_tensor("bvec32", [S, 1], fp32, kind="ExternalInput")
    out32 = nc.dram_tensor("out32", [S, BC], fp32, kind="ExternalOutput")

    NW = GB0 * BST            # N/NT tile width (max)
    GOFF = [sum(GSIZES[:i]) for i in range(NGRP)]

    with tile.TileContext(nc) as tc:
        loop_cm = tc.For_i(0, loop_n, 1) if loop_n > 1 else contextlib.nullcontext()
        with (
            loop_cm,
            tc.tile_pool(name="const", bufs=1) as const,
            tc.tile_pool(name="sentT_p", bufs=2) as sentT_p,
            tc.tile_pool(name="yt_p", bufs=2) as yt_p,
            tc.tile_pool(name="grp_p", bufs=2) as grp_p,
            tc.tile_pool(name="solve_p", bufs=5) as solve_p,
            tc.tile_pool(name="small", bufs=4) as small,
            tc.tile_pool(name="psmm", bufs=2, space="PSUM") as psmm,
            tc.tile_pool(name="psb2", bufs=2, space="PSUM") as psb2_p,
            tc.tile_pool(name="pssq", bufs=3, space="PSUM") as pssq_p,
            tc.tile_pool(name="pssv", bufs=1, space="PSUM") as pssv_p,
        ):
            wsim_sb = const.tile([128, HC, H], fp8)
            nc.sync.dma_start(wsim_sb[:], wsim8.ap())
            off_sb = const.tile([1, ROWS], fp16)
            nc.sync.dma_start(off_sb[:], off16h.ap())
            onesr_sb = const.tile([1, PADC], fp16)
            nc.sync.dma_start(onesr_sb[:], onesr16.ap())
            bvec_sb = const.tile([S, 1], fp32)
            nc.sync.dma_start(bvec_sb[:], bvec32.ap())

            st = {}          # per-group live tiles
            par = [0]        # drain engine parity

            def alt_copy(dst, src):
                if par[0] % 2 == 0:
                    nc.scalar.copy(dst, src)
                else:
                    nc.vector.tensor_copy(dst, src)
                par[0] += 1

            def load_thunk(g):
                gb = GSIZES[g]
                grows = gb * S
                def t():
                    sentT = sentT_p.tile([128, HC, SPAD], fp8, tag="sentT",
                                         name=f"sentT{g}")
                    st["sentT", g] = sentT
                    nc.gpsimd.memset(
                        sentT[:, :, grows : grows + PADC], 0.0
                    )
                    nc.sync.dma_start(
                        out=sentT[:, :, 0:grows],
                        in_=sent8.ap()[:, g, :, :],
                    )
                return [t]

            def mm1_thunks(g):
                gb = GSIZES[g]
                grows = gb * S
                def start():
                    yt = yt_p.tile([128, HC, SPAD], fp8, tag="yt",
                                   name=f"yt{g}")
                    st["yt", g] = yt
                    nc.gpsimd.memset(yt[:, :, grows : grows + PADC], 0.0)
                out = [start]

                def tile_t(n, m):
                    def t():
                        sentT = st["sentT", g]
                        yt = st["yt", g]
                        psy = psmm.tile([128, 512], fp32, tag="mm",
                                        name=f"psy{g}_{n}_{m}")
                        for tt in range(HP):
                            nc.tensor.matmul(
                                psy[:, :NT],
                                wsim_sb[:, 2 * tt : 2 * tt + 2,
                                        m * 128 : (m + 1) * 128],
                                sentT[:, 2 * tt : 2 * tt + 2, ts(n, NT)],
                                start=(tt == 0),
                                stop=(tt == HP - 1),
                                perf_mode=PM,
                            )
                        alt_copy(yt[:, m, ts(n, NT)], psy[:, :NT])
                    return t

                for n in range(grows // NT):
                    for m in range(HC):
                        out.append(tile_t(n, m))
                return out

            def ph2_thunks(g):
                gb = GSIZES[g]
                r0g = GOFF[g] * S
                def start():
                    N_cur = solve_p.tile([128, NW], fp16, tag="Nall",
                                         name=f"N0g{g}")
                    NT_cur = solve_p.tile([128, NW], fp16, tag="NTall",
                                          name=f"NT0g{g}")
                    st["N", g] = N_cur
                    st["NT", g] = NT_cur
                    st["svg", g] = grp_p.tile([S, GB0], fp16, tag="svg",
                                              name=f"svg{g}")
                    st["rg", g] = grp_p.tile([S, GB0], fp32, tag="rg",
                                             name=f"rg{g}")
                out = [start]

                def blk_t(blk):
                    def t():
                        sentT = st["sentT", g]
                        yt = st["yt", g]
                        NT_cur = st["NT", g]
                        svg, rg = st["svg", g], st["rg", g]
                        psb = psb2_p.tile([128, 512], fp32, tag="sim",
                                          name=f"sim{g}_{blk}")
                        # 12 DoubleRow matmuls into one bank, contiguous
                        # 100-col slots; single start clears the bank,
                        # per-element has_written handles the rest.
                        for q in range(4):
                            bl = blk * 4 + q
                            r0 = bl * S
                            dst = psb[:, q * S : q * S + S]
                            for tt in range(HP):
                                nc.tensor.matmul(
                                    dst,
                                    sentT[:, 2 * tt : 2 * tt + 2,
                                          r0 : r0 + PADC],
                                    yt[:, 2 * tt : 2 * tt + 2, r0 : r0 + S],
                                    start=(q == 0 and tt == 0),
                                    stop=False,
                                    perf_mode=PM,
                                )
                        # one K=1 off matmul over all 4 batches (400 cols)
                        nc.tensor.matmul(
                            psb[:, 0 : 4 * S],
                            onesr_sb[:],
                            off_sb[0:1, r0g + blk * 4 * S : r0g + (blk + 1) * 4 * S],
                            start=False,
                            stop=True,
                        )
                        qT4 = small.tile([S, 4 * S], fp16, tag="qT",
                                         name=f"qT{g}_{blk}")
                        nc.scalar.activation(
                            qT4[:],
                            psb[:S, 0 : 4 * S],
                            AF.Sigmoid,
                            bias=bvec_sb[:, 0:1],
                            scale=1.0 / WS,
                        )
                        with nc.allow_low_precision(reason="sv ~ 50, fp16 ok"):
                            nc.vector.reduce_sum(
                                out=svg[:, blk * 4 : blk * 4 + 4],
                                in_=qT4[:].rearrange("p (f w) -> p f w", w=S),
                                axis=X,
                            )
                        with nc.allow_low_precision(reason="1/sv fp16 ok"):
                            nc.vector.reciprocal(
                                rg[:, blk * 4 : blk * 4 + 4],
                                svg[:, blk * 4 : blk * 4 + 4],
                            )
                        for q in range(4):
                            bl = blk * 4 + q
                            nc.vector.tensor_scalar(
                                out=NT_cur[0:S, bl * BST : bl * BST + S],
                                in0=qT4[:, q * S : (q + 1) * S],
                                scalar1=rg[:, bl : bl + 1],
                                scalar2=LAMB,
                                op0=OP.mult,
                                op1=OP.mult,
                            )
                    return t

                out.extend(blk_t(blk) for blk in range(gb // 4))

                def transp():
                    # one group-wide transpose NT -> N
                    nc.sync.dma_start_transpose(
                        st["N", g][:, 0 : gb * BST].rearrange(
                            "p (n w) -> p n w", w=BST
                        )[:, :, 0:TRP],
                        st["NT", g][0:TRP, 0 : gb * BST],
                    )
                out.append(transp)

                def zset():
                    nc.vector.memset(
                        st["N", g][0:S, 0 : gb * BST].rearrange(
                            "p (n w) -> p n w", w=BST
                        )[:, :, S : S + 1],
                        Z0,
                    )
                out.append(zset)
                return out

            def solve_thunks(g):
                gb = GSIZES[g]
                packs = [(p, 4) for p in range(0, gb, 4)]
                out = []

                def lvl_start(j):
                    def t():
                        N_nxt = solve_p.tile([128, NW], fp16, tag="Nall",
                                             name=f"N{j + 1}g{g}")
                        NT_nxt = solve_p.tile([128, NW], fp16, tag="NTall",
                                              name=f"NT{j + 1}g{g}")
                        st["Nn", g] = N_nxt
                        st["NTn", g] = NT_nxt
                    return t

                def lvl_pack(j, p0, np_):
                    def t():
                        N_cur, NT_cur = st["N", g], st["NT", g]
                        N_nxt = st["Nn", g]
                        sq = pssq_p.tile([128, 512], fp32, tag="sq",
                                         name=f"sq{g}_{j}_{p0}")
                        for i in range(np_):
                            b = p0 + i
                            nc.tensor.matmul(
                                sq[:, i * BST : i * BST + S + 1],
                                NT_cur[0:S, b * BST : b * BST + PADC],
                                N_cur[0:S, b * BST : b * BST + S + 1],
                                start=True,
                                stop=True,
                            )
                        sq3 = sq[:S, :].rearrange("p (n w) -> p n w", w=BST)
                        dst3 = N_nxt[0:S, p0 * BST : (p0 + np_) * BST].rearrange(
                            "p (n w) -> p n w", w=BST
                        )
                        cur_z = N_cur[0:S, p0 * BST : (p0 + np_) * BST].rearrange(
                            "p (n w) -> p n w", w=BST
                        )[:, :, S : S + 1]
                        alt_copy(dst3[:, :, 0:S], sq3[:, 0:np_, 0:S])
                        nc.vector.tensor_tensor(
                            out=dst3[:, :, S : S + 1],
                            in0=sq3[:, 0:np_, S : S + 1],
                            in1=cur_z,
                            op=OP.add,
                        )
                    return t

                def lvl_packT(j, p0, np_):
                    def t():
                        # (N^2)^T = N^T @ N^T via matmul(lhsT=N, rhs=NT)
                        N_cur, NT_cur = st["N", g], st["NT", g]
                        NT_nxt = st["NTn", g]
                        sqT = pssq_p.tile([128, 512], fp32, tag="sq",
                                          name=f"sqT{g}_{j}_{p0}")
                        for i in range(np_):
                            b = p0 + i
                            nc.tensor.matmul(
                                sqT[:, i * BST : i * BST + S],
                                N_cur[0:S, b * BST : b * BST + PADC],
                                NT_cur[0:S, b * BST : b * BST + S],
                                start=True,
                                stop=True,
                            )
                        sq3 = sqT[:S, :].rearrange("p (n w) -> p n w", w=BST)
                        dst3 = NT_nxt[0:S, p0 * BST : (p0 + np_) * BST].rearrange(
                            "p (n w) -> p n w", w=BST
                        )
                        alt_copy(dst3[:, :, 0:S], sq3[:, 0:np_, 0:S])
                    return t

                def lvl_end(j):
                    def t():
                        st["N", g] = st["Nn", g]
                        st["NT", g] = st["NTn", g]
                    return t

                for j in range(NLEV):
                    out.append(lvl_start(j))
                    for p0, np_ in packs:
                        out.append(lvl_pack(j, p0, np_))
                        out.append(lvl_packT(j, p0, np_))
                    out.append(lvl_end(j))

                # matvec rounds: w1 = A z3, w2 = A w1, x = z3 + w1 + w2
                def mv1():
                    N_cur, NT_cur = st["N", g], st["NT", g]
                    w1p = pssv_p.tile([128, 512], fp32, tag="sv",
                                      name=f"w1p{g}")
                    for bl in range(gb):
                        nc.tensor.matmul(
                            w1p[:, bl : bl + 1],
                            NT_cur[0:S, bl * BST : bl * BST + PADC],
                            N_cur[0:S, bl * BST + S : bl * BST + S + 1],
                            start=True,
                            stop=True,
                        )
                    w1 = grp_p.tile([S, GB0], fp16, tag="w1",
                                    name=f"w1{g}")
                    st["w1", g] = w1
                    nc.vector.tensor_copy(w1[:, 0:gb], w1p[:S, 0:gb])
                    xt = grp_p.tile([S, GB0], fp32, tag="xt",
                                    name=f"xt{g}")
                    st["xt", g] = xt
                    zc = N_cur[0:S, 0 : gb * BST].rearrange(
                        "p (n w) -> p n w", w=BST
                    )
                    nc.vector.tensor_tensor(
                        out=xt[:, 0:gb].rearrange("p (n w) -> p n w", w=1),
                        in0=zc[:, :, S : S + 1],
                        in1=w1[:, 0:gb].rearrange("p (n w) -> p n w", w=1),
                        op=OP.add,
                    )
                out.append(mv1)

                def mv2():
                    NT_cur = st["NT", g]
                    w1 = st["w1", g]
                    xt = st["xt", g]
                    w2p = pssv_p.tile([128, 512], fp32, tag="sv",
                                      name=f"w2p{g}")
                    for bl in range(gb):
                        nc.tensor.matmul(
                            w2p[:, bl : bl + 1],
                            NT_cur[0:S, bl * BST : bl * BST + PADC],
                            w1[:, bl : bl + 1],
                            start=True,
                            stop=True,
                        )
                    xg = grp_p.tile([S, GB0], fp32, tag="xg",
                                    name=f"xg{g}")
                    nc.vector.tensor_tensor(
                        out=xg[:, 0:gb], in0=w2p[:S, 0:gb],
                        in1=xt[:, 0:gb], op=OP.add
                    )
                    nc.sync.dma_start(
                        out=out32.ap()[:, GOFF[g] : GOFF[g] + gb],
                        in_=xg[:, 0:gb],
                    )
                out.append(mv2)
                return out

            # ---- interleaved emission
            from collections import deque

            pending = deque()
            for g in range(NGRP):
                stream_a = []
                if g == 0:
                    stream_a += load_thunk(0)
                if g + 1 < NGRP:
                    stream_a += load_thunk(g + 1)
                stream_a += mm1_thunks(g)
                stream_a += ph2_thunks(g)
                for k, a in enumerate(stream_a):
                    a()
                    if k >= ILV_SKIP and pending:
                        for _ in range(ILV_RATE):
                            if pending:
                                pending.popleft()()
                pending.extend(solve_thunks(g))
            while pending:
                pending.popleft()()

    nc.compile()
    _CACHE[key] = nc
    return nc


def _prep(inputs):
    sent = np.ascontiguousarray(np.asarray(inputs["sent_vec"], dtype=np.float32))
    # [128, NGRP, HC, GROWS] per core, contiguous 1600B runs
    s_r = sent.reshape(NCORES, NGRP, GROWS, HC, 128)
    sent8 = np.ascontiguousarray(s_r.transpose(0, 4, 1, 3, 2)).astype(E4NP)
    W_rel = np.asarray(inputs["W_rel"], dtype=np.float32)
    W_cont = np.asarray(inputs["W_cont"], dtype=np.float32).reshape(H)
    sb = sent.reshape(B, S, H)
    d = sb.mean(axis=1)
    v = d @ W_rel.T + W_cont[None, :]
    off = np.matmul(sb, v[:, :, None])[:, :, 0]
    off16 = (off * WS).astype(np.float16).reshape(NCORES, 1, ROWS)
    W_sim = np.asarray(inputs["W_sim"], dtype=np.float32) * WS
    wsim8 = np.ascontiguousarray(
        W_sim.reshape(HC, 128, H).transpose(1, 0, 2)
    ).astype(E4NP)
    bval = float(np.asarray(inputs["b_matrix"]).reshape(-1)[0])
    onesr = np.ones((1, PADC), np.float16)
    bvec = np.full((S, 1), bval, np.float32)
    return [
        {
            "sent8": np.ascontiguousarray(sent8[i]),
            "wsim8": wsim8,
            "off16h": np.ascontiguousarray(off16[i]),
            "onesr16": onesr,
            "bvec32": bvec,
        }
        for i in range(NCORES)
    ]


def _run(in_maps, trace=False, **kw):
    from concourse.bass_utils import run_bass_kernel_spmd

    nc = _get_nc()
    return run_bass_kernel_spmd(nc, in_maps, list(range(NCORES)), trace=trace, **kw)


def kernel(**inputs):
    in_maps = _prep(inputs)
    res = _run(in_maps)
    out = np.concatenate([r["out32"].T for r in res.results], axis=0)
    return np.ascontiguousarray(out, dtype=np.float32)


if __name__ == "__main__":
    _get_nc()
    print("build ok")


# revision 10
# speedup vs baseline: 1.6911x; 1.3131x over previous
"""Trainium2 Bass kernel for nn_ExtSummarizer (B=512, S=100, H=768).

Math (per batch b, mask==1, true_dim==S):
  off[i] = s_i . v,  v = W_rel d + W_cont^T,  d = mean_i s_i   (host, fp32)
  q = sigmoid(s W_sim s^T + off[:,None] + b)
  sv[j] = sum_i q[i,j];  solve (I - lam*q*diag(1/sv)) x = y,  y = 1/S
  score = (1-lam) x

Device algorithm (transposed formulation, fp8 e4m3 matmuls):
  - mm1: yt[h',r] = sum_h (WS*W_sim)[h,h'] s[r,h] via fp8 DoubleRow
    (K=256/instr), fp32 PSUM, drained to fp8 yt (ACT/DVE alternating).
  - per 4-batch PSUM block (contiguous 400 cols of one bank): 12 fp8
    DoubleRow sim matmuls (single start on the first — per-element
    has_written gives each batch's columns overwrite-then-accumulate),
    then ONE K=1 fp16 matmul adds off over all 400 cols (stop=True).
    One ACT sigmoid -> qT fp16; DVE reduce -> sv (fp16); reciprocal;
    NT = lam*qT/sv (per-partition).  One group-wide DMA-xbar-transpose
    NT -> N after all blocks.
  - solve x = sum_{k<24} N^k z: 3 uniform doubling levels, each ONE
    matmul per batch: [N^2 | Nz+...] = N @ [N|z]; (N^2)^T via ONE
    group-wide DMA transpose per level.  Then w1=N^8 z3, w2=N^8 w1,
    x = z3+w1+w2.  fp16 operands.
  - N/NT stored in 128-col blocks per batch (z in col 100); garbage in
    unused rows/cols is never consumed by arithmetic.
  - emission interleaves solve(g) with mm1/phase2(g+1); 4 groups of 16.

Sharding: pure data parallel, 64 batches per core, 8 cores.
"""

import numpy as np
import ml_dtypes

B, S, H = 512, 100, 768
NCORES = 8
BC = B // NCORES          # 64 batches per core
ROWS = BC * S             # 6400 rows per core
LAMB = 0.8
GSIZES = [16, 16, 16, 16]
NGRP = len(GSIZES)
GB0 = 16                  # max group size (tile sizing)
GROWS = GB0 * S           # 1600 rows max per group
HC = H // 128             # 6 k-chunks
HP = HC // 2              # 3 DoubleRow k-pairs
NT = 400                  # mm1 moving-dim tile
BST = 128                 # N/NT per-batch block stride (z at col S)
NLEV = 1                  # doubling levels before the Perron tail-sum
# Perron tail: 1^T N = lam 1^T exactly (columns of q/sv sum to 1), and
# all non-Perron eigenvalues of N are O(lam/sqrt(S)).  After L doubling
# levels, x = z_L + N^(2^L) x with N^(2^L) x almost purely Perron, so
# x ~= z_L + w / (1 - lam^(2^L)),  w = N^(2^L) z_L.   (L=1: rel err 3e-4)
CTAIL = 1.0 / (1.0 - LAMB ** (2 ** NLEV))
WS = 16.0                 # fp8 scale on W_sim / off
Z0 = (1.0 - LAMB) / S
PADC = 128                # lhsT column width (FWL)
SPAD = GROWS + PADC
TRP = 112                 # DMA-transpose source rows (16-multiple >= S)
E4NP = ml_dtypes.float8_e4m3
ILV_SKIP = 0
ILV_RATE = 1

_CACHE = {}


def _get_nc(loop_n=1):
    key = ("nc", loop_n)
    if key in _CACHE:
        return _CACHE[key]

    import contextlib

    import concourse.mybir as mybir
    import concourse.tile as tile
    from concourse import bacc
    from concourse.bass import ts

    fp8 = mybir.dt.float8e4
    fp16 = mybir.dt.float16
    fp32 = mybir.dt.float32
    AF = mybir.ActivationFunctionType
    OP = mybir.AluOpType
    X = mybir.AxisListType.X
    PM = mybir.MatmulPerfMode.DoubleRow

    nc = bacc.Bacc(trn_type="TRN2", target_bir_lowering=False, debug=False)

    sent8 = nc.dram_tensor("sent8", [128, NGRP, HC, GROWS], fp8,
                           kind="ExternalInput")
    wsim8 = nc.dram_tensor("wsim8", [128, HC, H], fp8, kind="ExternalInput")
    off16h = nc.dram_tensor("off16h", [1, ROWS], fp16, kind="ExternalInput")
    onesr16 = nc.dram_tensor("onesr16", [1, PADC], fp16, kind="ExternalInput")
    bvec32 = nc.dram_tensor("bvec32", [S, 1], fp32, kind="ExternalInput")
    out32 = nc.dram_tensor("out32", [S, BC], fp32, kind="ExternalOutput")

    NW = GB0 * BST            # N/NT tile width (max)
    GOFF = [sum(GSIZES[:i]) for i in range(NGRP)]

    with tile.TileContext(nc) as tc:
        loop_cm = tc.For_i(0, loop_n, 1) if loop_n > 1 else contextlib.nullcontext()
        with (
            loop_cm,
            tc.tile_pool(name="const", bufs=1) as const,
            tc.tile_pool(name="sentT_p", bufs=2) as sentT_p,
            tc.tile_pool(name="yt_p", bufs=2) as yt_p,
            tc.tile_pool(name="grp_p", bufs=2) as grp_p,
            tc.tile_pool(name="solve_p", bufs=5) as solve_p,
            tc.tile_pool(name="small", bufs=4) as small,
            tc.tile_pool(name="psmm", bufs=2, space="PSUM") as psmm,
            tc.tile_pool(name="psb2", bufs=2, space="PSUM") as psb2_p,
            tc.tile_pool(name="pssq", bufs=3, space="PSUM") as pssq_p,
            tc.tile_pool(name="pssv", bufs=1, space="PSUM") as pssv_p,
        ):
            wsim_sb = const.tile([128, HC, H], fp8)
            nc.sync.dma_start(wsim_sb[:], wsim8.ap())
            off_sb = const.tile([1, ROWS], fp16)
            nc.sync.dma_start(off_sb[:], off16h.ap())
            onesr_sb = const.tile([1, PADC], fp16)
            nc.sync.dma_start(onesr_sb[:], onesr16.ap())
            bvec_sb = const.tile([S, 1], fp32)
            nc.sync.dma_start(bvec_sb[:], bvec32.ap())

            st = {}          # per-group live tiles
            par = [0]        # drain engine parity

            def alt_copy(dst, src):
                if par[0] % 2 == 0:
                    nc.scalar.copy(dst, src)
                else:
                    nc.vector.tensor_copy(dst, src)
                par[0] += 1

            def load_thunk(g):
                gb = GSIZES[g]
                grows = gb * S
                def t():
                    sentT = sentT_p.tile([128, HC, SPAD], fp8, tag="sentT",
                                         name=f"sentT{g}")
                    st["sentT", g] = sentT
                    nc.gpsimd.memset(
                        sentT[:, :, grows : grows + PADC], 0.0
                    )
                    nc.sync.dma_start(
                        out=sentT[:, :, 0:grows],
                        in_=sent8.ap()[:, g, :, :],
                    )
                return [t]

            def mm1_thunks(g):
                gb = GSIZES[g]
                grows = gb * S
                def start():
                    yt = yt_p.tile([128, HC, SPAD], fp8, tag="yt",
                                   name=f"yt{g}")
                    st["yt", g] = yt
                    nc.gpsimd.memset(yt[:, :, grows : grows + PADC], 0.0)
                out = [start]

                def tile_t(n, m):
                    def t():
                        sentT = st["sentT", g]
                        yt = st["yt", g]
                        psy = psmm.tile([128, 512], fp32, tag="mm",
                                        name=f"psy{g}_{n}_{m}")
                        for tt in range(HP):
                            nc.tensor.matmul(
                                psy[:, :NT],
                                wsim_sb[:, 2 * tt : 2 * tt + 2,
                                        m * 128 : (m + 1) * 128],
                                sentT[:, 2 * tt : 2 * tt + 2, ts(n, NT)],
                                start=(tt == 0),
                                stop=(tt == HP - 1),
                                perf_mode=PM,
                            )
                        alt_copy(yt[:, m, ts(n, NT)], psy[:, :NT])
                    return t

                for n in range(grows // NT):
                    for m in range(HC):
                        out.append(tile_t(n, m))
                return out

            def ph2_thunks(g):
                gb = GSIZES[g]
                r0g = GOFF[g] * S
                def start():
                    N_cur = solve_p.tile([128, NW], fp16, tag="Nall",
                                         name=f"N0g{g}")
                    NT_cur = solve_p.tile([128, NW], fp16, tag="NTall",
                                          name=f"NT0g{g}")
                    st["N", g] = N_cur
                    st["NT", g] = NT_cur
                    st["svg", g] = grp_p.tile([S, GB0], fp16, tag="svg",
                                              name=f"svg{g}")
                    st["rg", g] = grp_p.tile([S, GB0], fp32, tag="rg",
                                             name=f"rg{g}")
                out = [start]

                def blk_t(blk):
                    def t():
                        sentT = st["sentT", g]
                        yt = st["yt", g]
                        NT_cur = st["NT", g]
                        svg, rg = st["svg", g], st["rg", g]
                        psb = psb2_p.tile([128, 512], fp32, tag="sim",
                                          name=f"sim{g}_{blk}")
                        # 12 DoubleRow matmuls into one bank, contiguous
                        # 100-col slots; single start clears the bank,
                        # per-element has_written handles the rest.
                        for q in range(4):
                            bl = blk * 4 + q
                            r0 = bl * S
                            dst = psb[:, q * S : q * S + S]
                            for tt in range(HP):
                                nc.tensor.matmul(
                                    dst,
                                    sentT[:, 2 * tt : 2 * tt + 2,
                                          r0 : r0 + PADC],
                                    yt[:, 2 * tt : 2 * tt + 2, r0 : r0 + S],
                                    start=(q == 0 and tt == 0),
                                    stop=False,
                                    perf_mode=PM,
                                )
                        # one K=1 off matmul over all 4 batches (400 cols)
                        nc.tensor.matmul(
                            psb[:, 0 : 4 * S],
                            onesr_sb[:],
                            off_sb[0:1, r0g + blk * 4 * S : r0g + (blk + 1) * 4 * S],
                            start=False,
                            stop=True,
                        )
                        qT4 = small.tile([S, 4 * S], fp16, tag="qT",
                                         name=f"qT{g}_{blk}")
                        nc.scalar.activation(
                            qT4[:],
                            psb[:S, 0 : 4 * S],
                            AF.Sigmoid,
                            bias=bvec_sb[:, 0:1],
                            scale=1.0 / WS,
                        )
                        with nc.allow_low_precision(reason="sv ~ 50, fp16 ok"):
                            nc.vector.reduce_sum(
                                out=svg[:, blk * 4 : blk * 4 + 4],
                                in_=qT4[:].rearrange("p (f w) -> p f w", w=S),
                                axis=X,
                            )
                        with nc.allow_low_precision(reason="1/sv fp16 ok"):
                            nc.vector.reciprocal(
                                rg[:, blk * 4 : blk * 4 + 4],
                                svg[:, blk * 4 : blk * 4 + 4],
                            )
                        for q in range(4):
                            bl = blk * 4 + q
                            nc.vector.tensor_scalar(
                                out=NT_cur[0:S, bl * BST : bl * BST + S],
                                in0=qT4[:, q * S : (q + 1) * S],
                                scalar1=rg[:, bl : bl + 1],
                                scalar2=LAMB,
                                op0=OP.mult,
                                op1=OP.mult,
                            )
                    return t

                out.extend(blk_t(blk) for blk in range(gb // 4))

                def transp():
                    # one group-wide transpose NT -> N
                    nc.sync.dma_start_transpose(
                        st["N", g][:, 0 : gb * BST].rearrange(
                            "p (n w) -> p n w", w=BST
                        )[:, :, 0:TRP],
                        st["NT", g][0:TRP, 0 : gb * BST],
                    )
                out.append(transp)

                def zset():
                    nc.vector.memset(
                        st["N", g][0:S, 0 : gb * BST].rearrange(
                            "p (n w) -> p n w", w=BST
                        )[:, :, S : S + 1],
                        Z0,
                    )
                out.append(zset)
                return out

            def solve_thunks(g):
                gb = GSIZES[g]
                packs = [(p, 4) for p in range(0, gb, 4)]
                out = []

                def lvl_start(j):
                    def t():
                        N_nxt = solve_p.tile([128, NW], fp16, tag="Nall",
                                             name=f"N{j + 1}g{g}")
                        NT_nxt = solve_p.tile([128, NW], fp16, tag="NTall",
                                              name=f"NT{j + 1}g{g}")
                        st["Nn", g] = N_nxt
                        st["NTn", g] = NT_nxt
                    return t

                def lvl_pack(j, p0, np_):
                    def t():
                        N_cur, NT_cur = st["N", g], st["NT", g]
                        N_nxt = st["Nn", g]
                        sq = pssq_p.tile([128, 512], fp32, tag="sq",
                                         name=f"sq{g}_{j}_{p0}")
                        for i in range(np_):
                            b = p0 + i
                            nc.tensor.matmul(
                                sq[:, i * BST : i * BST + S + 1],
                                NT_cur[0:S, b * BST : b * BST + PADC],
                                N_cur[0:S, b * BST : b * BST + S + 1],
                                start=True,
                                stop=True,
                            )
                        sq3 = sq[:S, :].rearrange("p (n w) -> p n w", w=BST)
                        dst3 = N_nxt[0:S, p0 * BST : (p0 + np_) * BST].rearrange(
                            "p (n w) -> p n w", w=BST
                        )
                        cur_z = N_cur[0:S, p0 * BST : (p0 + np_) * BST].rearrange(
                            "p (n w) -> p n w", w=BST
                        )[:, :, S : S + 1]
                        alt_copy(dst3[:, :, 0:S], sq3[:, 0:np_, 0:S])
                        nc.vector.tensor_tensor(
                            out=dst3[:, :, S : S + 1],
                            in0=sq3[:, 0:np_, S : S + 1],
                            in1=cur_z,
                            op=OP.add,
                        )
                    return t

                def lvl_packT(j, p0, np_):
                    def t():
                        # (N^2)^T = N^T @ N^T via matmul(lhsT=N, rhs=NT)
                        N_cur, NT_cur = st["N", g], st["NT", g]
                        NT_nxt = st["NTn", g]
                        sqT = pssq_p.tile([128, 512], fp32, tag="sq",
                                          name=f"sqT{g}_{j}_{p0}")
                        for i in range(np_):
                            b = p0 + i
                            nc.tensor.matmul(
                                sqT[:, i * BST : i * BST + S],
                                N_cur[0:S, b * BST : b * BST + PADC],
                                NT_cur[0:S, b * BST : b * BST + S],
                                start=True,
                                stop=True,
                            )
                        sq3 = sqT[:S, :].rearrange("p (n w) -> p n w", w=BST)
                        dst3 = NT_nxt[0:S, p0 * BST : (p0 + np_) * BST].rearrange(
                            "p (n w) -> p n w", w=BST
                        )
                        alt_copy(dst3[:, :, 0:S], sq3[:, 0:np_, 0:S])
                    return t

                def lvl_end(j):
                    def t():
                        st["N", g] = st["Nn", g]
                        st["NT", g] = st["NTn", g]
                    return t

                for j in range(NLEV):
                    out.append(lvl_start(j))
                    for p0, np_ in packs:
                        out.append(lvl_pack(j, p0, np_))
                        out.append(lvl_packT(j, p0, np_))
                    out.append(lvl_end(j))

                # w = N_L z_L, then x = z_L + CTAIL * w (Perron tail-sum)
                def mv1():
                    N_cur, NT_cur = st["N", g], st["NT", g]
                    w1p = pssv_p.tile([128, 512], fp32, tag="sv",
                                      name=f"w1p{g}")
                    for bl in range(gb):
                        nc.tensor.matmul(
                            w1p[:, bl : bl + 1],
                            NT_cur[0:S, bl * BST : bl * BST + PADC],
                            N_cur[0:S, bl * BST + S : bl * BST + S + 1],
                            start=True,
                            stop=True,
                        )
                    xt = grp_p.tile([S, GB0], fp32, tag="xt",
                                    name=f"xt{g}")
                    nc.vector.tensor_scalar(
                        out=xt[:, 0:gb],
                        in0=w1p[:S, 0:gb],
                        scalar1=CTAIL,
                        scalar2=None,
                        op0=OP.mult,
                    )
                    xg = grp_p.tile([S, GB0], fp32, tag="xg",
                                    name=f"xg{g}")
                    zc = N_cur[0:S, 0 : gb * BST].rearrange(
                        "p (n w) -> p n w", w=BST
                    )
                    nc.vector.tensor_tensor(
                        out=xg[:, 0:gb].rearrange("p (n w) -> p n w", w=1),
                        in0=zc[:, :, S : S + 1],
                        in1=xt[:, 0:gb].rearrange("p (n w) -> p n w", w=1),
                        op=OP.add,
                    )
                    nc.sync.dma_start(
                        out=out32.ap()[:, GOFF[g] : GOFF[g] + gb],
                        in_=xg[:, 0:gb],
                    )
                out.append(mv1)
                return out

            # ---- interleaved emission
            from collections import deque

            pending = deque()
            for g in range(NGRP):
                stream_a = []
                if g == 0:
                    stream_a += load_thunk(0)
                if g + 1 < NGRP:
                    stream_a += load_thunk(g + 1)
                stream_a += mm1_thunks(g)
                stream_a += ph2_thunks(g)
                for k, a in enumerate(stream_a):
                    a()
                    if k >= ILV_SKIP and pending:
                        for _ in range(ILV_RATE):
                            if pending:
                                pending.popleft()()
                pending.extend(solve_thunks(g))
            while pending:
                pending.popleft()()

    nc.compile()
    _CACHE[key] = nc
    return nc


def _prep(inputs):
    sent = np.ascontiguousarray(np.asarray(inputs["sent_vec"], dtype=np.float32))
    # [128, NGRP, HC, GROWS] per core, contiguous 1600B runs
    s_r = sent.reshape(NCORES, NGRP, GROWS, HC, 128)
    sent8 = np.ascontiguousarray(s_r.transpose(0, 4, 1, 3, 2)).astype(E4NP)
    W_rel = np.asarray(inputs["W_rel"], dtype=np.float32)
    W_cont = np.asarray(inputs["W_cont"], dtype=np.float32).reshape(H)
    sb = sent.reshape(B, S, H)
    d = sb.mean(axis=1)
    v = d @ W_rel.T + W_cont[None, :]
    off = np.matmul(sb, v[:, :, None])[:, :, 0]
    off16 = (off * WS).astype(np.float16).reshape(NCORES, 1, ROWS)
    W_sim = np.asarray(inputs["W_sim"], dtype=np.float32) * WS
    wsim8 = np.ascontiguousarray(
        W_sim.reshape(HC, 128, H).transpose(1, 0, 2)
    ).astype(E4NP)
    bval = float(np.asarray(inputs["b_matrix"]).reshape(-1)[0])
    onesr = np.ones((1, PADC), np.float16)
    bvec = np.full((S, 1), bval, np.float32)
    return [
        {
            "sent8": np.ascontiguousarray(sent8[i]),
            "wsim8": wsim8,
            "off16h": np.ascontiguousarray(off16[i]),
            "onesr16": onesr,
            "bvec32": bvec,
        }
        for i in range(NCORES)
    ]


def _run(in_maps, trace=False, **kw):
    from concourse.bass_utils import run_bass_kernel_spmd

    nc = _get_nc()
    return run_bass_kernel_spmd(nc, in_maps, list(range(NCORES)), trace=trace, **kw)


def kernel(**inputs):
    in_maps = _prep(inputs)
    res = _run(in_maps)
    out = np.concatenate([r["out32"].T for r in res.results], axis=0)
    return np.ascontiguousarray(out, dtype=np.float32)


if __name__ == "__main__":
    _get_nc()
    print("build ok")
